# revision 11
# baseline (speedup 1.0000x reference)
"""Single attention head (B=8, S=2048, D=768, H=12) on 8 TRN2 NeuronCores.

Data-parallel over batch (1 element/core). v2 design:
  - Host prep is layout only: per-batch permutation packing masked-in keys
    first (key extent compacts 2048 -> T_pad ~ 1152), x transposed to
    [128, chunk, ko, 512] fp32 for contiguous DMA, weights packed
    [Wk | Wq/sqrt(H) | Wv] fp32, additive bias rows for the mask.
  - QKV projection in ONE fp32r pass (fp32r matmuls stream at fp16 rate for
    moving dims >= 256 and carry ~1.5e-4 relative error, which this
    near-one-hot softmax tolerates; measured output rel err ~1e-3).
  - Pass A (row max, [s,t] orientation): fp16 q*k + bias via a 13-row
    matmul per s-tile, DVE reduce_max per 512-slab. Only needs +-11.
  - Pass B ([t,s] orientation): fp32r 14-row matmul (12 q rows + bias row +
    "-max" row), pairs of t-tiles share a [128,1024] PSUM tile so ACT exp
    runs 1024 wide -> fp16 p tiles.
  - PV: fp16, column-tiled 2 ways (M=32 at array cols 0/64) so two t-tiles
    stream concurrently; denominator rides along as a ones-column; DVE adds
    the two column-group partials.
  - Max-row and output transposes ride the DMA xbar (dma_start_transpose)
    or one tiny PE transpose per chunk, keeping the PE for real matmuls.
"""

import math
import os

import numpy as np

B, S, D, H = 8, 2048, 768, 12
N_CORES = 8
NCH = 4            # s chunks
SCH = S // NCH     # 512
BIAS_A = -60000.0  # fp16 additive mask bias (pass A)
BIAS_B = -1.0e8    # fp32 additive mask bias (pass B)


def _build(nc_mod, T_pad):
    bass, mybir, tile, bacc = nc_mod
    f32 = mybir.dt.float32
    f32r = mybir.dt.float32r
    f16 = mybir.dt.float16
    AF = mybir.ActivationFunctionType
    X = mybir.AxisListType.X

    NT = T_pad // 128
    # pass-A slabs over t: full 512s plus a remainder (multiple of 128)
    slabs = [(o, 512) for o in range(0, T_pad - 511, 512)]
    if T_pad % 512:
        slabs.append((T_pad - T_pad % 512, T_pad % 512))
    NSL = len(slabs)
    last_cov = (T_pad - 1) // SCH   # last chunk whose qkv covers kTb columns

    nc = bacc.Bacc("TRN2", target_bir_lowering=False, debug=False,
                   num_devices=N_CORES)

    x_ext = nc.dram_tensor("x", [128, NCH * 6 * SCH], f32r,
                           kind="ExternalInput")
    w_ext = nc.dram_tensor("w", [128, 6 * 76], f32r, kind="ExternalInput")
    biasA_ext = nc.dram_tensor("biasA", [1, T_pad], f16, kind="ExternalInput")
    onesT_ext = nc.dram_tensor("onesT", [1, T_pad], f16, kind="ExternalInput")
    constB_ext = nc.dram_tensor("constB", [2, T_pad], f32r,
                                kind="ExternalInput")
    onesS_ext = nc.dram_tensor("onesS", [1, S], f32r, kind="ExternalInput")
    out_ext = nc.dram_tensor("out", [128, 256], f32, kind="ExternalOutput")
    dbg_maxc = nc.dram_tensor("dbg_maxc", [128, 16], f32, kind="ExternalOutput")
    dbg_rhsB = nc.dram_tensor("dbg_rhsB", [16, S], f32, kind="ExternalOutput")
    dbg_kTb = nc.dram_tensor("dbg_kTb", [16, T_pad], f32, kind="ExternalOutput")
    dbg_qA = nc.dram_tensor("dbg_qA", [13, S], f32, kind="ExternalOutput")
    dbg_vcomb = nc.dram_tensor("dbg_vcomb", [32, S], f32, kind="ExternalOutput")
    dbg_vaug = nc.dram_tensor("dbg_vaug", [128, NT * 16], f32, kind="ExternalOutput")
    dbg_p = nc.dram_tensor("dbg_p", [128, 1024], f32, kind="ExternalOutput")
    dbg_vaugT = nc.dram_tensor("dbg_vaugT", [32, T_pad], f32, kind="ExternalOutput")

    from concourse.masks import make_identity

    with tile.TileContext(nc) as tc:
        with tc.tile_pool(name="sb", bufs=1) as sb, \
             tc.tile_pool(name="pp", bufs=3) as ppool, \
             tc.tile_pool(name="qkvp", bufs=1, space="PSUM") as qkvp, \
             tc.tile_pool(name="ap", bufs=2, space="PSUM") as ap, \
             tc.tile_pool(name="bp", bufs=2, space="PSUM") as bp, \
             tc.tile_pool(name="vp", bufs=1, space="PSUM") as vp:

            xsb = sb.tile([128, 6, S], f32r)
            w = sb.tile([128, 6, 76], f32r)
            kTb = sb.tile([16, T_pad], f32r)    # 0-11 k, 12 bias, 13 = -1
            rhsB = sb.tile([16, S], f32r)       # 0-11 q, 12 = 1, 13 = m
            qA = sb.tile([13, S], f16)          # 0-11 q fp16, 12 = 1
            kA = sb.tile([13, T_pad], f16)      # 0-11 k fp16, 12 = biasA
            vaugT = sb.tile([32, T_pad], f16)   # 0-11 v, 12 = 1, rest 0
            vaug = sb.tile([128, NT, 16], f16)
            ident = sb.tile([128, 128], f32)
            ident16 = sb.tile([16, 16], f16)
            maxh = sb.tile([128, 16, 4], f32)
            maxc = sb.tile([128, 16], f32)
            negmT = sb.tile([4, 128], f32r)
            vcomb = sb.tile([32, S], f16)       # 0-12 combined out+denom
            vstage = sb.tile([16, S], f32)
            rec4 = sb.tile([128, 16], f32)
            outsb = sb.tile([128, 16, 16], f32)

            nc.gpsimd.memset(qA[:, :], 1.0)       # row 12 stays 1
            nc.gpsimd.memset(vaugT[:, :], 0.0)    # rows 13-31 stay 0
            nc.gpsimd.memset(vcomb[:, :], 0.0)    # rows 13-31 stay 0
            make_identity(nc, ident[:])
            make_identity(nc, ident16[:])

            nc.sync.dma_start(w[:], w_ext.ap().rearrange(
                "p (ko m) -> p ko m", ko=6))
            nc.sync.dma_start(kA[12:13, :], biasA_ext.ap())
            nc.sync.dma_start(kTb[12:14, :], constB_ext.ap())
            nc.sync.dma_start(rhsB[12:13, :], onesS_ext.ap())
            nc.sync.dma_start(vaugT[12:13, :], onesT_ext.ap())
            xr = x_ext.ap().rearrange("p (c ko s) -> p c ko s", c=NCH, ko=6)
            for c in range(NCH):
                cs = slice(c * SCH, (c + 1) * SCH)
                nc.sync.dma_start(xsb[:, :, cs], xr[:, c])

            # ---- QKV projection (fp32r), one pass ----
            for c in range(NCH):
                cs = slice(c * SCH, (c + 1) * SCH)
                qkv = qkvp.tile([76, SCH], f32, tag="qkv")
                for ko in range(6):
                    nc.tensor.matmul(qkv[:, :], w[:, ko, :], xsb[:, ko, cs],
                                     start=(ko == 0), stop=(ko == 5))
                # egress: q rows (fp32r for pass B, fp16 for pass A)
                nc.scalar.copy(rhsB[0:12, cs], qkv[32:44, :])
                nc.scalar.copy(qA[0:12, cs], qkv[32:44, :])
                if c * SCH < T_pad:
                    t0 = c * SCH
                    t1 = min((c + 1) * SCH, T_pad)
                    tsl = slice(0, t1 - t0)
                    ts = slice(t0, t1)
                    nc.scalar.copy(kTb[0:12, ts], qkv[0:12, tsl])
                    nc.scalar.copy(kA[0:12, ts], qkv[0:12, tsl])
                    nc.scalar.copy(vaugT[0:12, ts], qkv[64:76, tsl])
                if c == last_cov:
                    for j in range(NT):
                        vt = ap.tile([128, 512], f16, name=f"vt{j}",
                                     tag="pa")
                        nc.tensor.transpose(
                            vt[:, 0:16], vaugT[0:16, j * 128:(j + 1) * 128],
                            ident16[:])
                        nc.vector.tensor_copy(vaug[:, j, 0:16], vt[:, 0:16])

            # ---- attention pipeline ----
            def emit_A(c):
                for stl in range(4):
                    st = 4 * c + stl
                    s0 = st * 128
                    for si, (to, tw) in enumerate(slabs):
                        at = ap.tile([128, 512], f32, tag="pa")
                        nc.tensor.matmul(
                            at[:, 0:tw], qA[0:13, s0:s0 + 128],
                            kA[0:13, to:to + tw], start=True, stop=True)
                        nc.vector.reduce_max(
                            maxh[:, st, si:si + 1], at[:, 0:tw], axis=X)
                    nc.vector.reduce_max(
                        maxc[:, st:st + 1], maxh[:, st, 0:NSL], axis=X)

            def emit_negm(c):
                c4 = slice(4 * c, 4 * c + 4)
                mt = ap.tile([128, 512], f32, tag="pa")
                nc.tensor.transpose(mt[0:4, 0:128], maxc[:, c4], ident[:])
                nc.scalar.copy(negmT[:, :], mt[0:4, 0:128])
                for k in range(4):
                    col = c * SCH + k * 128
                    nc.sync.dma_start(rhsB[13:14, col:col + 128],
                                      negmT[k:k + 1, :])

            npair = (NT + 1) // 2
            g_last = {0: 2 * (npair - 1)}
            g_last[1] = 2 * ((NT - 2) // 2) + 1 if NT >= 2 else -1

            def emit_B_PV(c):
                cs = slice(c * SCH, (c + 1) * SCH)
                vacc = vp.tile([96, SCH], f32, tag="v")
                for jp in range(npair):
                    j0, j1 = 2 * jp, 2 * jp + 1
                    width = 1024 if j1 < NT else 512
                    bt = bp.tile([128, 1024], f32, tag="b")
                    nc.tensor.matmul(
                        bt[:, 0:512], kTb[0:14, j0 * 128:(j0 + 1) * 128],
                        rhsB[0:14, cs], start=True, stop=True)
                    if j1 < NT:
                        nc.tensor.matmul(
                            bt[:, 512:1024], kTb[0:14, j1 * 128:(j1 + 1) * 128],
                            rhsB[0:14, cs], start=True, stop=True)
                    p = ppool.tile([128, 1024], f16, tag="p")
                    nc.scalar.activation(p[:, 0:width], bt[:, 0:width], AF.Exp)
                    if c == 0 and jp == 0:
                        dbgp = sb.tile([128, 1024], f32, name="dbgp")
                        nc.vector.tensor_copy(dbgp[:], p[:])
                        nc.sync.dma_start(dbg_p.ap(), dbgp[:])
                    nc.tensor.matmul(
                        vacc[0:16, :], vaug[:, j0, 0:16], p[:, 0:512],
                        start=(j0 == 0), stop=(j0 == g_last[0]),
                        tile_position=(0, 0))
                    if j1 < NT:
                        nc.tensor.matmul(
                            vacc[64:80, :], vaug[:, j1, 0:16], p[:, 512:1024],
                            start=(j1 == 1), stop=(j1 == g_last[1]),
                            tile_position=(0, 64))
                nc.scalar.copy(vstage[0:16, cs], vacc[64:80, :])
                nc.vector.tensor_add(vcomb[0:16, cs], vacc[0:16, :],
                                     vstage[0:16, cs])

            def emit_out(c):
                for stl in range(4):
                    st = 4 * c + stl
                    ot = ap.tile([128, 512], f16, name=f"ot{st}", tag="pa")
                    nc.tensor.transpose(
                        ot[:, 0:16], vcomb[0:16, st * 128:(st + 1) * 128],
                        ident16[:])
                    nc.vector.reciprocal(rec4[:, st:st + 1], ot[:, 12:13])
                    nc.vector.tensor_scalar_mul(
                        outsb[:, st, 0:12], ot[:, 0:12], rec4[:, st:st + 1])

            emit_A(0)
            emit_negm(0)
            for c in range(NCH):
                emit_B_PV(c)
                if c + 1 < NCH:
                    emit_A(c + 1)
                    emit_negm(c + 1)
                emit_out(c)

            nc.sync.dma_start(
                out_ext.ap(), outsb[:].rearrange("p a b -> p (a b)"))
            dbg1 = sb.tile([128, 16], f32)
            nc.vector.tensor_copy(dbg1[:], maxc[:])
            nc.sync.dma_start(dbg_maxc.ap(), dbg1[:])
            dbg2 = sb.tile([16, S], f32)
            nc.vector.tensor_copy(dbg2[:], rhsB[:])
            nc.sync.dma_start(dbg_rhsB.ap(), dbg2[:])
            dbg3 = sb.tile([16, T_pad], f32)
            nc.vector.tensor_copy(dbg3[:], kTb[:])
            nc.sync.dma_start(dbg_kTb.ap(), dbg3[:])
            dbg4 = sb.tile([13, S], f32)
            nc.vector.tensor_copy(dbg4[:], qA[:])
            nc.sync.dma_start(dbg_qA.ap(), dbg4[:])
            dbg5 = sb.tile([32, S], f32)
            nc.vector.tensor_copy(dbg5[:], vcomb[:])
            nc.sync.dma_start(dbg_vcomb.ap(), dbg5[:])
            dbg6 = sb.tile([128, NT, 16], f32)
            nc.vector.tensor_copy(dbg6[:], vaug[:])
            nc.sync.dma_start(dbg_vaug.ap(), dbg6[:].rearrange("p a b -> p (a b)"))
            dbg7 = sb.tile([32, T_pad], f32)
            nc.vector.tensor_copy(dbg7[:], vaugT[:])
            nc.sync.dma_start(dbg_vaugT.ap(), dbg7[:])

    nc.compile()
    return nc


def kernel(x, mask, key_weight, query_weight, value_weight):
    import concourse.bass as bass
    import concourse.mybir as mybir
    import concourse.tile as tile
    from concourse import bacc, bass_utils

    x = np.asarray(x, dtype=np.float32)
    mask = np.asarray(mask)
    wk = np.asarray(key_weight, dtype=np.float32)
    wq = np.asarray(query_weight, dtype=np.float32)
    wv = np.asarray(value_weight, dtype=np.float32)

    w2 = np.zeros((D, 76), dtype=np.float32)
    w2[:, 0:12] = wk
    w2[:, 32:44] = wq / math.sqrt(H)
    w2[:, 64:76] = wv
    w_dev = np.ascontiguousarray(
        w2.reshape(6, 128, 76).transpose(1, 0, 2)).reshape(128, 6 * 76)

    perms, nbs = [], []
    for b in range(B):
        m = mask[b, 0].astype(np.int64)
        perm = np.argsort(1 - m, kind="stable")
        perms.append(perm)
        nbs.append(int(m.sum()))
    T_pad = max(128, int(np.ceil(max(max(nbs), 1) / 128.0)) * 128)
    T_pad = min(T_pad, S)

    in_maps = []
    for b in range(B):
        xp = x[b][perms[b]]                    # [S, D]
        xp = xp.reshape(NCH, SCH, 6, 128)      # [c, s, ko, p]
        x_dev = np.ascontiguousarray(
            xp.transpose(3, 0, 2, 1)).reshape(128, NCH * 6 * SCH)
        biasA = np.zeros((1, T_pad), dtype=np.float16)
        biasA[0, nbs[b]:] = BIAS_A
        constB = np.zeros((2, T_pad), dtype=np.float32)
        constB[0, nbs[b]:] = BIAS_B
        constB[1, :] = -1.0
        in_maps.append({"x": x_dev, "w": w_dev, "biasA": biasA,
                        "constB": constB,
                        "onesS": np.ones((1, S), dtype=np.float32),
                        "onesT": np.ones((1, T_pad), dtype=np.float16)})

    import time as _time
    _t0 = _time.time()
    print(f"[kernel] building graph, T_pad={T_pad}", flush=True)
    nc = _build((bass, mybir, tile, bacc), T_pad)
    print(f"[kernel] graph+bacc compile done in {_time.time() - _t0:.1f}s",
          flush=True)

    trace = os.environ.get("BASS_KERNEL_TRACE", "0") == "1"
    if trace:
        import sys
        import types
        from trn_agent_boot.trn_boot import _ntff_profile_via_ctypes
        hook = _ntff_profile_via_ctypes("/opt/axon/libaxon_pjrt.so")
        m = types.ModuleType("antenv.axon_hooks")
        m.get_axon_ntff_profile_hook = lambda: hook
        sys.modules["antenv.axon_hooks"] = m
        bass_utils.upload_artifacts = lambda tmpdir: "local://" + tmpdir

    res = bass_utils.run_bass_kernel_spmd(
        nc, in_maps, core_ids=list(range(N_CORES)), trace=trace)
    if trace:
        print(f"HW exec time: {res.exec_time_ns} ns", flush=True)

    out = np.empty((B, S, H), dtype=np.float32)
    for b in range(B):
        o = res.results[b]["out"].reshape(128, 16, 16)[:, :, :H]
        out[b, perms[b], :] = o.transpose(1, 0, 2).reshape(S, H)
    return out


# revision 12
# speedup vs baseline: 1.0171x; 1.0171x over previous
"""Single attention head (B=8, S=2048, D=768, H=12) on 8 TRN2 NeuronCores.

Data-parallel over batch (1 element/core). v2 design:
  - Host prep is layout only: per-batch permutation packing masked-in keys
    first (key extent compacts 2048 -> T_pad ~ 1152), x transposed to
    [128, chunk, ko, 512] fp32 for contiguous DMA, weights packed
    [Wk | Wq/sqrt(H) | Wv] fp32, additive bias rows for the mask.
  - QKV projection in ONE fp32r pass (fp32r matmuls stream at fp16 rate for
    moving dims >= 256 and carry ~1.5e-4 relative error, which this
    near-one-hot softmax tolerates; measured output rel err ~1e-3).
  - Pass A (row max, [s,t] orientation): fp16 q*k + bias via a 13-row
    matmul per s-tile, DVE reduce_max per 512-slab. Only needs +-11.
  - Pass B ([t,s] orientation): fp32r 14-row matmul (12 q rows + bias row +
    "-max" row), pairs of t-tiles share a [128,1024] PSUM tile so ACT exp
    runs 1024 wide -> fp16 p tiles.
  - PV: fp16, column-tiled 2 ways (M=32 at array cols 0/64) so two t-tiles
    stream concurrently; denominator rides along as a ones-column; DVE adds
    the two column-group partials.
  - Max-row and output transposes ride the DMA xbar (dma_start_transpose)
    or one tiny PE transpose per chunk, keeping the PE for real matmuls.
"""

import math
import os

import numpy as np

B, S, D, H = 8, 2048, 768, 12
N_CORES = 8
NCH = 4            # s chunks
SCH = S // NCH     # 512
BIAS_A = -60000.0  # fp16 additive mask bias (pass A)
BIAS_B = -1.0e8    # fp32 additive mask bias (pass B)


def _build(nc_mod, T_pad):
    bass, mybir, tile, bacc = nc_mod
    f32 = mybir.dt.float32
    f32r = mybir.dt.float32r
    f16 = mybir.dt.float16
    AF = mybir.ActivationFunctionType
    X = mybir.AxisListType.X

    NT = T_pad // 128
    # pass-A slabs over t: full 512s plus a remainder (multiple of 128)
    slabs = [(o, 512) for o in range(0, T_pad - 511, 512)]
    if T_pad % 512:
        slabs.append((T_pad - T_pad % 512, T_pad % 512))
    NSL = len(slabs)
    last_cov = (T_pad - 1) // SCH   # last chunk whose qkv covers kTb columns

    nc = bacc.Bacc("TRN2", target_bir_lowering=False, debug=False,
                   num_devices=N_CORES)

    x_ext = nc.dram_tensor("x", [128, NCH * 6 * SCH], f32r,
                           kind="ExternalInput")
    w_ext = nc.dram_tensor("w", [128, 6 * 76], f32r, kind="ExternalInput")
    biasA_ext = nc.dram_tensor("biasA", [1, T_pad], f16, kind="ExternalInput")
    onesT_ext = nc.dram_tensor("onesT", [1, T_pad], f16, kind="ExternalInput")
    constB_ext = nc.dram_tensor("constB", [2, T_pad], f32r,
                                kind="ExternalInput")
    onesS_ext = nc.dram_tensor("onesS", [1, S], f32r, kind="ExternalInput")
    out_ext = nc.dram_tensor("out", [128, 256], f32, kind="ExternalOutput")

    from concourse.masks import make_identity

    with tile.TileContext(nc) as tc:
        with tc.tile_pool(name="sb", bufs=1) as sb, \
             tc.tile_pool(name="pp", bufs=3) as ppool, \
             tc.tile_pool(name="qkvp", bufs=1, space="PSUM") as qkvp, \
             tc.tile_pool(name="ap", bufs=2, space="PSUM") as ap, \
             tc.tile_pool(name="bp", bufs=2, space="PSUM") as bp, \
             tc.tile_pool(name="vp", bufs=1, space="PSUM") as vp:

            xsb = sb.tile([128, 6, S], f32r)
            w = sb.tile([128, 6, 76], f32r)
            kTb = sb.tile([16, T_pad], f32r)    # 0-11 k, 12 bias, 13 = -1
            rhsB = sb.tile([16, S], f32r)       # 0-11 q, 12 = 1, 13 = m
            qA = sb.tile([13, S], f16)          # 0-11 q fp16, 12 = 1
            kA = sb.tile([13, T_pad], f16)      # 0-11 k fp16, 12 = biasA
            vaugT = sb.tile([32, T_pad], f16)   # 0-11 v, 12 = 1, rest 0
            vaug = sb.tile([128, NT, 16], f16)
            ident = sb.tile([128, 128], f32)
            ident16 = sb.tile([16, 16], f16)
            maxh = sb.tile([128, 16, 4], f32)
            maxc = sb.tile([128, 16], f32)
            negmT = sb.tile([4, 128], f32r)
            vcomb = sb.tile([32, S], f16)       # 0-12 combined out+denom
            vstage = sb.tile([16, S], f32)
            rec4 = sb.tile([128, 16], f32)
            outsb = sb.tile([128, 16, 16], f32)

            nc.gpsimd.memset(qA[:, :], 1.0)       # row 12 stays 1
            nc.gpsimd.memset(vaugT[:, :], 0.0)    # rows 13-31 stay 0
            nc.gpsimd.memset(vcomb[:, :], 0.0)    # rows 13-31 stay 0
            make_identity(nc, ident[:])
            make_identity(nc, ident16[:])

            nc.sync.dma_start(w[:], w_ext.ap().rearrange(
                "p (ko m) -> p ko m", ko=6))
            nc.sync.dma_start(kA[12:13, :], biasA_ext.ap())
            nc.sync.dma_start(kTb[12:14, :], constB_ext.ap())
            nc.sync.dma_start(rhsB[12:13, :], onesS_ext.ap())
            nc.sync.dma_start(vaugT[12:13, :], onesT_ext.ap())
            xr = x_ext.ap().rearrange("p (c ko s) -> p c ko s", c=NCH, ko=6)
            for c in range(NCH):
                cs = slice(c * SCH, (c + 1) * SCH)
                nc.sync.dma_start(xsb[:, :, cs], xr[:, c])

            # ---- QKV projection (fp32r), one pass ----
            for c in range(NCH):
                cs = slice(c * SCH, (c + 1) * SCH)
                qkv = qkvp.tile([76, SCH], f32, tag="qkv")
                for ko in range(6):
                    nc.tensor.matmul(qkv[:, :], w[:, ko, :], xsb[:, ko, cs],
                                     start=(ko == 0), stop=(ko == 5))
                # egress: q rows (fp32r for pass B, fp16 for pass A)
                nc.scalar.copy(rhsB[0:12, cs], qkv[32:44, :])
                nc.scalar.copy(qA[0:12, cs], qkv[32:44, :])
                if c * SCH < T_pad:
                    t0 = c * SCH
                    t1 = min((c + 1) * SCH, T_pad)
                    tsl = slice(0, t1 - t0)
                    ts = slice(t0, t1)
                    nc.scalar.copy(kTb[0:12, ts], qkv[0:12, tsl])
                    nc.scalar.copy(kA[0:12, ts], qkv[0:12, tsl])
                    nc.scalar.copy(vaugT[0:12, ts], qkv[64:76, tsl])
                if c == last_cov:
                    for j in range(NT):
                        vt = ap.tile([128, 512], f16, name=f"vt{j}",
                                     tag="pa")
                        nc.tensor.transpose(
                            vt[:, 0:16], vaugT[0:16, j * 128:(j + 1) * 128],
                            ident16[:])
                        nc.vector.tensor_copy(vaug[:, j, 0:16], vt[:, 0:16])

            # ---- attention pipeline ----
            def emit_A(c):
                for stl in range(4):
                    st = 4 * c + stl
                    s0 = st * 128
                    for si, (to, tw) in enumerate(slabs):
                        at = ap.tile([128, 512], f32, tag="pa")
                        nc.tensor.matmul(
                            at[:, 0:tw], qA[0:13, s0:s0 + 128],
                            kA[0:13, to:to + tw], start=True, stop=True)
                        nc.vector.reduce_max(
                            maxh[:, st, si:si + 1], at[:, 0:tw], axis=X)
                    nc.vector.reduce_max(
                        maxc[:, st:st + 1], maxh[:, st, 0:NSL], axis=X)

            def emit_negm(c):
                c4 = slice(4 * c, 4 * c + 4)
                mt = ap.tile([128, 512], f32, tag="pa")
                nc.tensor.transpose(mt[0:4, 0:128], maxc[:, c4], ident[:])
                nc.scalar.copy(negmT[:, :], mt[0:4, 0:128])
                for k in range(4):
                    col = c * SCH + k * 128
                    nc.sync.dma_start(rhsB[13:14, col:col + 128],
                                      negmT[k:k + 1, :])

            npair = (NT + 1) // 2
            g_last = {0: 2 * (npair - 1)}
            g_last[1] = 2 * ((NT - 2) // 2) + 1 if NT >= 2 else -1

            def emit_B_PV(c):
                cs = slice(c * SCH, (c + 1) * SCH)
                vacc = vp.tile([96, SCH], f32, tag="v")
                for jp in range(npair):
                    j0, j1 = 2 * jp, 2 * jp + 1
                    width = 1024 if j1 < NT else 512
                    bt = bp.tile([128, 1024], f32, tag="b")
                    nc.tensor.matmul(
                        bt[:, 0:512], kTb[0:14, j0 * 128:(j0 + 1) * 128],
                        rhsB[0:14, cs], start=True, stop=True)
                    if j1 < NT:
                        nc.tensor.matmul(
                            bt[:, 512:1024], kTb[0:14, j1 * 128:(j1 + 1) * 128],
                            rhsB[0:14, cs], start=True, stop=True)
                    p = ppool.tile([128, 1024], f16, tag="p")
                    nc.scalar.activation(p[:, 0:width], bt[:, 0:width], AF.Exp)
                    nc.tensor.matmul(
                        vacc[0:16, :], vaug[:, j0, 0:16], p[:, 0:512],
                        start=(j0 == 0), stop=(j0 == g_last[0]),
                        tile_position=(0, 0))
                    if j1 < NT:
                        nc.tensor.matmul(
                            vacc[64:80, :], vaug[:, j1, 0:16], p[:, 512:1024],
                            start=(j1 == 1), stop=(j1 == g_last[1]),
                            tile_position=(0, 64))
                nc.scalar.copy(vstage[0:16, cs], vacc[64:80, :])
                nc.vector.tensor_add(vcomb[0:16, cs], vacc[0:16, :],
                                     vstage[0:16, cs])

            def emit_out(c):
                for stl in range(4):
                    st = 4 * c + stl
                    ot = ap.tile([128, 512], f16, name=f"ot{st}", tag="pa")
                    nc.tensor.transpose(
                        ot[:, 0:16], vcomb[0:16, st * 128:(st + 1) * 128],
                        ident16[:])
                    nc.vector.reciprocal(rec4[:, st:st + 1], ot[:, 12:13])
                    nc.vector.tensor_scalar_mul(
                        outsb[:, st, 0:12], ot[:, 0:12], rec4[:, st:st + 1])

            emit_A(0)
            emit_negm(0)
            for c in range(NCH):
                emit_B_PV(c)
                if c + 1 < NCH:
                    emit_A(c + 1)
                    emit_negm(c + 1)
                emit_out(c)

            nc.sync.dma_start(
                out_ext.ap(), outsb[:].rearrange("p a b -> p (a b)"))
    nc.compile()
    return nc


def kernel(x, mask, key_weight, query_weight, value_weight):
    import concourse.bass as bass
    import concourse.mybir as mybir
    import concourse.tile as tile
    from concourse import bacc, bass_utils

    x = np.asarray(x, dtype=np.float32)
    mask = np.asarray(mask)
    wk = np.asarray(key_weight, dtype=np.float32)
    wq = np.asarray(query_weight, dtype=np.float32)
    wv = np.asarray(value_weight, dtype=np.float32)

    w2 = np.zeros((D, 76), dtype=np.float32)
    w2[:, 0:12] = wk
    w2[:, 32:44] = wq / math.sqrt(H)
    w2[:, 64:76] = wv
    w_dev = np.ascontiguousarray(
        w2.reshape(6, 128, 76).transpose(1, 0, 2)).reshape(128, 6 * 76)

    perms, nbs = [], []
    for b in range(B):
        m = mask[b, 0].astype(np.int64)
        perm = np.argsort(1 - m, kind="stable")
        perms.append(perm)
        nbs.append(int(m.sum()))
    T_pad = max(128, int(np.ceil(max(max(nbs), 1) / 128.0)) * 128)
    T_pad = min(T_pad, S)

    in_maps = []
    for b in range(B):
        xp = x[b][perms[b]]                    # [S, D]
        xp = xp.reshape(NCH, SCH, 6, 128)      # [c, s, ko, p]
        x_dev = np.ascontiguousarray(
            xp.transpose(3, 0, 2, 1)).reshape(128, NCH * 6 * SCH)
        biasA = np.zeros((1, T_pad), dtype=np.float16)
        biasA[0, nbs[b]:] = BIAS_A
        constB = np.zeros((2, T_pad), dtype=np.float32)
        constB[0, nbs[b]:] = BIAS_B
        constB[1, :] = -1.0
        in_maps.append({"x": x_dev, "w": w_dev, "biasA": biasA,
                        "constB": constB,
                        "onesS": np.ones((1, S), dtype=np.float32),
                        "onesT": np.ones((1, T_pad), dtype=np.float16)})

    import time as _time
    _t0 = _time.time()
    print(f"[kernel] building graph, T_pad={T_pad}", flush=True)
    nc = _build((bass, mybir, tile, bacc), T_pad)
    print(f"[kernel] graph+bacc compile done in {_time.time() - _t0:.1f}s",
          flush=True)

    trace = os.environ.get("BASS_KERNEL_TRACE", "0") == "1"
    if trace:
        import sys
        import types
        from trn_agent_boot.trn_boot import _ntff_profile_via_ctypes
        hook = _ntff_profile_via_ctypes("/opt/axon/libaxon_pjrt.so")
        m = types.ModuleType("antenv.axon_hooks")
        m.get_axon_ntff_profile_hook = lambda: hook
        sys.modules["antenv.axon_hooks"] = m
        bass_utils.upload_artifacts = lambda tmpdir: "local://" + tmpdir

    res = bass_utils.run_bass_kernel_spmd(
        nc, in_maps, core_ids=list(range(N_CORES)), trace=trace)
    if trace:
        print(f"HW exec time: {res.exec_time_ns} ns", flush=True)

    out = np.empty((B, S, H), dtype=np.float32)
    for b in range(B):
        o = res.results[b]["out"].reshape(128, 16, 16)[:, :, :H]
        out[b, perms[b], :] = o.transpose(1, 0, 2).reshape(S, H)
    return out
